# revision 32
# baseline (speedup 1.0000x reference)
"""CapsuleLayer kernel for Trainium2 (8 NeuronCores, data-parallel over batch).

Math: the reference's dynamic-routing loop is degenerate — `delta` is summed
over the capsule axis and broadcast back, so the logits stay constant across
capsules and softmax stays uniform (1/16) for all 3 iterations. The module
therefore reduces exactly to

    t   = (conv2d(x, sum_c W[c]) + sum_c b[c]) / 16      # 16-out-channel conv
    out = sign(t) * t^2 / (1 + t^2)                      # scalar squash

The capsule sum is folded into the conv weights on the host (conv is linear in
the weights), leaving a [O=16, I=64, 3, 3] VALID conv + pointwise epilogue.

Device strategy per core (8 images per core, one image PAIR per wave p;
the shipped config is KERNEL_OPTS = chain_opt + epi_mode="dve" + ham_warm=8):
  - x feed: 5 DMAs on the SP HWDGE ring (pair 0 split into rows [0,34) and
    [34,66), pairs 1-3 whole [128,66,66] bf16 ~1.1 MB each). The metric is
    the single-shot critical chain (the bench loop is an all-engine barrier
    per iteration), so the front matters: first chunk ready ~2.2us + ~2us
    sem receipt.
  - The conv runs on the TensorEngine as 9-tap accumulating matmuls packed
    8-wide into the 128x128 array with tile_position (2 row groups = image
    parity rg, 4 col groups j). Each 32-wide column group carries TWO
    h-tiles: per (tap, rg, j) two matmuls whose [64,32] stationary blocks
    are zero-masked on opposite 16-column halves. Zeros accumulate
    harmlessly into the other half's PSUM partitions, so ALL 128 PSUM
    partitions hold valid conv outputs:
      partition = 32*j + 16*half + o   (h-tile ht = j+4*half, out channel o)
      free      = 512*rg + 64*hr + w   (hr = row within h-tile, w = col)
    The tap loop runs half-OUTER (all 9 taps of half 0 first): under the
    ht=j+4*half remap half-0 matmuls read only x rows 0..33, so the PE
    starts ~1.5us earlier on feed chunk A while chunk B streams.
    PSUM tile per pair: [128, 1024] f32 = 2 banks. Measured PE time:
    19.6us/iter = 72 groups x 272ns (216 stream + 26.7 LDWEIGHTS + ~28
    stagger); every matmul self-loads its stationary (no walrus dedup —
    measured), so this is the structural floor for M=16 out channels vs
    the 32-column tile granularity.
  - 8 dummy matmuls (garbage into pair-0's PSUM, overwritten by its real
    start=True group) run at the top of each iteration to warm the PE HAM
    clock gate (cold = 1.2 GHz) during the x-feed front.
  - Epilogue per pair, balanced across ScalarE and DVE so PSUM is released
    early and ScalarE (the old 3-act bottleneck) does only 2 ops:
      u  = Square(ps + b)            [ScalarE, f32]
      g  = (ps + b >= 0) in {0,1}    [DVE tensor_scalar, reads PSUM]
      r  = Reciprocal(u + 1)         [ScalarE, bf16; raw InstActivation —
                                      the bass wrapper refuses this func;
                                      accuracy verified by test.py]
      sn = 1 - 2g = -sign(t+b)       [DVE tensor_scalar, bf16 4x]
      f  = (r - 1) * sn              [DVE scalar_tensor_tensor, bf16 2x]
    f == sign(t+b) * (t+b)^2/(1+(t+b)^2).  Square and Reciprocal share act
    table set 13 (preseeded) — no per-iteration table loads.
  - Outs: pairs 0-2 as whole [128,1024] bf16 DMAs on the SP ring (idle
    after the feed; receipts overlap later work). Pair 3 — the exposed
    tail — is computed AND dumped as two FD=512 half-chains so the
    Sq->Rec->f->DMA->HBM-receipt hops of the two halves pipeline.
  - Host unshuffles the [128, 4, 1024] dumps (gather_out(chain_opt=True)).
"""

import numpy as np

N_CORES = 8
B_PER_CORE = 8  # 64 images / 8 cores


def _act_raw(nc, out, in_, func, bias, scale=1.0):
    """nc.scalar.activation without the Reciprocal/Rsqrt ValueError guard.
    bias/scale must be floats (imm) for Copy/Reciprocal."""
    import concourse.mybir as mybir

    se = nc.scalar
    ins = [se.lower_ap(in_)]
    for arg in (bias, scale, 0.0):
        ins.append(mybir.ImmediateValue(dtype=mybir.dt.float32, value=float(arg)))
    return se.add_instruction(
        mybir.InstActivation(
            name=se.bass.get_next_instruction_name(),
            func=func,
            ins=ins,
            outs=[se.lower_ap(out)],
        )
    )


def _build_nc(
    repeat=1,
    loop_repeat=1,
    conv_bf16=False,
    parts=None,
    x_bufs=4,
    ps_bufs=4,
    recip_act=True,
    feed_probe=None,  # None | "one" (single big DMA) | int n (only n pair DMAs)
    mm_order="jrg",  # "jrg": j innermost (baseline) | "rgj": rg innermost
    epi_mode="act",  # "act": 3 ScalarE ops | "dve": 2 ScalarE + 3 DVE ops
    out_eng="split",  # "split": outs on SP+ACT | "act": all outs on ACT ring
    fp_bufs=4,
    wk_bufs=2,
    chain_opt=False,  # single-shot critical-chain optimizations:
    #  - h-tile remap ht=j+4*half so half-0 matmuls touch only x rows 0..33
    #  - pair-0 x DMA split into rows [0,34) + [34,66) for an earlier PE start
    #  - outs p0-p2 on the SP ring (idle after the x feed), p3 on ACT
    ham_warm=0,  # N dummy matmuls (N=512 free) at the top of each iteration
    # to bridge the PE-idle gap (front+tail) so the HAM clock gate stays at
    # 2.4 GHz instead of re-throttling to 1.2 GHz every iteration
    mm_same_w=False,  # bench-only: all matmuls use the same stationary slice
    # (wrong math) to probe whether repeated LDWEIGHTS get elided
    mm_m16=False,  # half-0 matmuls use a true [64,16] stationary (LDWEIGHTS
    # cost is P cols / 1.2GHz, so 13.3ns instead of 26.7ns) writing only
    # their 16 PSUM partitions; half-1 keeps the 32-wide zero-masked form
    # and is emitted FIRST so its start=True bank-clear precedes half-0 data
):
    # parts: subset of {"in", "mm", "epi", "out"} for bench attribution;
    # None = all.
    if parts is None:
        parts = {"in", "mm", "epi", "out"}
    import contextlib

    import concourse.bacc as bacc
    import concourse.mybir as mybir
    import concourse.tile as tile

    f32 = mybir.dt.float32
    cdt = mybir.dt.bfloat16 if conv_bf16 else f32
    # Bacc (not raw Bass): its finalize() runs move_matmul_waits_to_ldweights
    # + generate_event_semaphores, required for TRN2's 1-wait-per-instruction
    # limit (our first matmuls collect several Tile sem waits).
    nc = bacc.Bacc(None, target_bir_lowering=False, debug=False)

    x_d = nc.dram_tensor("x", [512, 66, 66], cdt, kind="ExternalInput")
    w_d = nc.dram_tensor("w", [128, 576], cdt, kind="ExternalInput")
    bv_d = nc.dram_tensor("bvec", [128, 1], f32, kind="ExternalInput")
    # Raw dump [partition, pair, 512*rg + 64*hr + w] (partition-major so the
    # out DMA writes 8KB contiguous per partition); unshuffled on the host.
    out_d = nc.dram_tensor("out", [128, 4, 1024], cdt, kind="ExternalOutput")

    AF = mybir.ActivationFunctionType

    with tile.TileContext(nc) as tc:
        with (
            tc.tile_pool(name="const", bufs=1) as cp,
            tc.tile_pool(name="xp", bufs=x_bufs) as xp,
            tc.tile_pool(name="psp", bufs=ps_bufs, space="PSUM") as psp,
            tc.tile_pool(name="pscp", bufs=1, space="PSUM") as pscp,
            tc.tile_pool(name="wk", bufs=wk_bufs) as wk,
            tc.tile_pool(name="fp", bufs=fp_bufs) as fp,
        ):
            # Constants ride the ACT ring so the SP ring can start streaming
            # x(p0) at t=0 at full HBM bandwidth (single-shot critical path).
            w_t = cp.tile([128, 576], cdt)
            nc.scalar.dma_start(out=w_t[:, :], in_=w_d[:, :])
            b_t = cp.tile([128, 1], f32)
            nc.scalar.dma_start(out=b_t[:, :], in_=bv_d[:, :])
            nb_t = cp.tile([128, 1], f32)
            # Pre-seed the reciprocal_and_small act table set (id 13: holds
            # Square/Sign/Copy/Reciprocal — everything this kernel uses) so
            # insert_act_table_loads doesn't alternate set loads (~2.7us
            # each) inside the loop between a Square/Sign set and the
            # Reciprocal-only set.
            nc.scalar.add_instruction(
                mybir.InstLoadActFuncSet(
                    name=nc.get_next_instruction_name(),
                    act_func_set_id=13,
                    ins=[],
                    outs=[],
                )
            )
            # nb = -b (one-time)
            nc.scalar.activation(nb_t[:, :], b_t[:, :], AF.Copy, bias=0.0, scale=-1.0)

            if loop_repeat > 1:  # bench only: HW loop repeating the body
                loop_cm = tc.For_i(
                    0,
                    loop_repeat,
                    1,
                    hint_engines=(
                        mybir.EngineType.PE,
                        mybir.EngineType.Activation,
                        mybir.EngineType.DVE,
                        mybir.EngineType.SP,
                    ),
                )
            else:
                loop_cm = contextlib.nullcontext()
            # Bench-attribution support: when a stage's producer is excluded,
            # back the consumed tiles with memset'd pre-loop tiles so CoreSim
            # doesn't see read-before-write.
            pre_x = None
            if "mm" in parts and "in" not in parts:
                pre_x = [cp.tile([128, 66, 66], cdt, name=f"xs{p}") for p in range(4)]
                for t_ in pre_x:
                    nc.vector.memset(t_[:, :, :], 0.0)
            pre_ps = None
            if "epi" in parts and "mm" not in parts:
                pre_ps = [pscp.tile([128, 1024], f32, name=f"pss{p}") for p in range(4)]
                for t_ in pre_ps:
                    nc.vector.memset(t_[:, :], 0.0)
            with loop_cm:
                if parts == {"cal"}:
                    cal_t = wk.tile([128, 16], f32, tag="cal")
                    nc.vector.memset(cal_t[:, :], 0.0)
                for it in range(0 if parts == {"cal"} else repeat):
                    # Per-pair x DMAs, staggered across the two HWDGE rings
                    # (SP: p0,p2; ACT: p1,p3 after the small const loads), so
                    # mm(p0) starts ~5us in while later pairs stream behind.
                    # All four x DMAs on the SP ring IN ORDER: one ring's FIFO
                    # is the only way to get staggered completions (SDMA
                    # engines round-robin across queues at packet granularity,
                    # so DMAs on different rings all finish together). p0
                    # lands ~5us in and mm(p0) starts while p1..p3 stream.
                    xt4 = []
                    if pre_x is not None:
                        xt4 = pre_x
                    elif feed_probe == "one":
                        xf = xp.tile([128, 4, 66, 66], cdt, tag="x", name="xft")
                        if "in" in parts:
                            nc.sync.dma_start(
                                out=xf[:, :, :, :],
                                in_=x_d[:, :, :].rearrange(
                                    "(pp q) hh ww -> q pp hh ww", pp=4
                                ),
                            )
                        xt4 = [xf[:, p] for p in range(4)]
                    else:
                        npair = feed_probe if isinstance(feed_probe, int) else 4
                        for p in range(4):
                            x1 = xp.tile([128, 66, 66], cdt, tag="x", name="x1t")
                            xt4.append(x1)
                            if "in" in parts and p < npair:
                                if chain_opt and p == 0:
                                    # split pair 0 so the first-emitted
                                    # half's matmuls can start ~1.5us early.
                                    # With mm_m16 half 1 (rows 32..65) goes
                                    # first (its start=True M=32 clear must
                                    # precede half-0 data); else half 0
                                    # (rows 0..33) first.
                                    if mm_m16:
                                        nc.sync.dma_start(
                                            out=x1[:, 32:66, :],
                                            in_=x_d[0:128, 32:66, :],
                                        )
                                        nc.sync.dma_start(
                                            out=x1[:, 0:32, :],
                                            in_=x_d[0:128, 0:32, :],
                                        )
                                    else:
                                        nc.sync.dma_start(
                                            out=x1[:, 0:34, :],
                                            in_=x_d[0:128, 0:34, :],
                                        )
                                        nc.sync.dma_start(
                                            out=x1[:, 34:66, :],
                                            in_=x_d[0:128, 34:66, :],
                                        )
                                else:
                                    nc.sync.dma_start(
                                        out=x1[:, :, :],
                                        in_=x_d[128 * p : 128 * (p + 1), :, :],
                                    )
                    ps0 = None
                    if ham_warm and "mm" in parts and pre_ps is None:
                        # Dummy matmuls (garbage math into a region that the
                        # real pair-0 matmuls overwrite with start=True) to
                        # keep the PE busy across the front/tail idle gap so
                        # the HAM clock gate stays at 2.4 GHz.
                        ps0 = psp.tile([128, 1024], f32, tag="ps", name="ps0")
                        for _ in range(ham_warm):
                            nc.tensor.matmul(
                                ps0[0:32, 0:512],
                                w_t[0:64, 0:32],
                                w_t[0:64, 0:512],
                                start=True,
                                stop=True,
                                tile_position=(0, 0),
                                skip_group_check=True,
                            )
                    fout = []
                    for p in range(4):
                        x_t = xt4[p]
                        if pre_ps is not None:
                            ps = pre_ps[p]
                        elif p == 0 and ps0 is not None:
                            ps = ps0
                        else:
                            ps = psp.tile([128, 1024], f32, tag="ps")

                        if "mm" in parts:
                            # chain_opt: half OUTER so all 9 taps of half 0
                            # (which under the ht=j+4*half remap read only x
                            # rows 0..33, i.e. the first feed chunk) issue
                            # before any half-1 matmul waits on chunk B.
                            if chain_opt and mm_m16:
                                # half 1 first: its M=32 start=True clear
                                # must precede half 0's M=16 writes
                                th_iter = [(t, half) for half in (1, 0)
                                           for t in range(9)]
                            elif chain_opt:
                                th_iter = [(t, half) for half in range(2)
                                           for t in range(9)]
                            else:
                                th_iter = [(t, half) for t in range(9)
                                           for half in range(2)]
                            for t, half in th_iter:
                                kh, kw = divmod(t, 3)
                                if True:
                                    if mm_order == "jrg":
                                        combos = [(rg, j) for rg in range(2)
                                                  for j in range(4)]
                                    else:  # rg innermost: adjacent MMs
                                        # alternate row groups so LDWEIGHTS
                                        # can pull ahead of in-flight MMs
                                        combos = [(rg, j) for j in range(4)
                                                  for rg in range(2)]
                                    for rg, j in combos:
                                            ht = (j + 4 * half) if chain_opt \
                                                else (2 * j + half)
                                            h0 = ht * 8
                                            m16 = mm_m16 and half == 0
                                            wc0 = 32 * (2 * t + half)
                                            if mm_same_w:
                                                w_ap = w_t[
                                                    64 * rg : 64 * rg + 64,
                                                    0 : (16 if m16 else 32),
                                                ]
                                            else:
                                                w_ap = w_t[
                                                    64 * rg : 64 * rg + 64,
                                                    wc0 : wc0 + (16 if m16
                                                                 else 32),
                                                ]
                                            if mm_m16:
                                                st = (t == 0 and half == 1)
                                                sp = (t == 8)
                                            else:
                                                st = (t == 0 and half == 0)
                                                sp = (t == 8 and half == 1)
                                            nc.tensor.matmul(
                                                ps[
                                                    32 * j : 32 * j
                                                    + (16 if m16 else 32),
                                                    512 * rg : 512 * rg + 512,
                                                ],
                                                w_ap,
                                                x_t[
                                                    64 * rg : 64 * rg + 64,
                                                    h0 + kh : h0 + kh + 8,
                                                    kw : kw + 64,
                                                ],
                                                start=st,
                                                stop=sp,
                                                tile_position=(64 * rg, 32 * j),
                                                skip_group_check=True,
                                            )

                        if ("epi" in parts and epi_mode == "dve"
                                and chain_opt and p == 3):
                            # Last pair: split the epilogue into two FD=512
                            # half-chains so the serial tail (Sq -> Rec -> f
                            # -> out DMA -> HBM write receipt) pipelines and
                            # the first half's out overlaps the second half.
                            fh = []
                            for s in range(2):
                                sl = slice(512 * s, 512 * s + 512)
                                u = wk.tile([128, 512], f32, tag=f"u3{s}")
                                g = wk.tile([128, 512], cdt, tag=f"g3{s}")
                                sn = wk.tile([128, 512], cdt, tag=f"sn3{s}")
                                r = wk.tile([128, 512], cdt, tag=f"r3{s}")
                                nc.scalar.activation(
                                    u[:, :], ps[:, sl], AF.Square,
                                    bias=b_t[:, 0:1],
                                )
                                nc.vector.tensor_scalar(
                                    g[:, :], ps[:, sl], b_t[:, 0:1], 0.0,
                                    mybir.AluOpType.add, mybir.AluOpType.is_ge,
                                )
                                _act_raw(nc, r[:, :], u[:, :], AF.Reciprocal,
                                         1.0)
                                nc.vector.tensor_scalar(
                                    sn[:, :], g[:, :], -2.0, 1.0,
                                    mybir.AluOpType.mult,
                                    mybir.AluOpType.add,
                                )
                                f = fp.tile([128, 512], cdt, tag=f"f3{s}")
                                nc.vector.scalar_tensor_tensor(
                                    f[:, :], r[:, :], 1.0, sn[:, :],
                                    mybir.AluOpType.subtract,
                                    mybir.AluOpType.mult,
                                )
                                fh.append(f)
                            fout.append((p, tuple(fh)))
                        elif "epi" in parts and epi_mode == "dve":
                            # ScalarE 2 ops + DVE 3 ops (balance the engines;
                            # release ps as early as possible for the
                            # loop-carried PSUM-pool barrier):
                            #   u  = Square(ps + b)            [ScalarE]
                            #   g  = (ps >= 0) in {0,1}        [DVE, reads ps]
                            #   r  = Reciprocal(u + 1)         [ScalarE]
                            #   sn = 1 - 2g = -sign(t+b)       [DVE]
                            #   f  = (r - 1) * sn              [DVE]
                            u = wk.tile([128, 1024], f32, tag="u")
                            g = wk.tile([128, 1024], cdt, tag="g")
                            sn = wk.tile([128, 1024], cdt, tag="sn")
                            r = wk.tile([128, 1024], cdt, tag="r")
                            nc.scalar.activation(
                                u[:, :], ps[:, :], AF.Square, bias=b_t[:, 0:1]
                            )
                            nc.vector.tensor_scalar(
                                g[:, :], ps[:, :], b_t[:, 0:1], 0.0,
                                mybir.AluOpType.add, mybir.AluOpType.is_ge,
                            )
                            _act_raw(nc, r[:, :], u[:, :], AF.Reciprocal, 1.0)
                            nc.vector.tensor_scalar(
                                sn[:, :], g[:, :], -2.0, 1.0,
                                mybir.AluOpType.mult, mybir.AluOpType.add,
                            )
                            f = fp.tile([128, 1024], cdt, tag="f")
                            nc.vector.scalar_tensor_tensor(
                                f[:, :], r[:, :], 1.0, sn[:, :],
                                mybir.AluOpType.subtract, mybir.AluOpType.mult,
                            )
                            fout.append((p, f))
                        elif "epi" in parts:
                            u = wk.tile([128, 1024], f32, tag="u")
                            sn = wk.tile([128, 1024], cdt, tag="sn")
                            r = wk.tile([128, 1024], cdt, tag="r")
                            nc.scalar.activation(
                                u[:, :], ps[:, :], AF.Square, bias=b_t[:, 0:1]
                            )
                            # sn = sign(-(t+b)) = -sign(t+b)
                            nc.scalar.activation(
                                sn[:, :], ps[:, :], AF.Sign,
                                bias=nb_t[:, 0:1], scale=-1.0,
                            )
                            if recip_act:
                                # r = 1/(1+u) via the ScalarE spline table
                                _act_raw(nc, r[:, :], u[:, :], AF.Reciprocal, 1.0)
                            else:
                                w1 = wk.tile([128, 1024], f32, tag="w1")
                                rf = wk.tile([128, 1024], f32, tag="rf")
                                nc.vector.tensor_scalar_add(w1[:, :], u[:, :], 1.0)
                                nc.vector.reciprocal_approx_fast(rf[:, :], w1[:, :])
                                nc.vector.tensor_copy(r[:, :], rf[:, :])
                            # f = (r-1)*sn = sign(t+b)*(1 - r)
                            f = fp.tile([128, 1024], cdt, tag="f")
                            nc.vector.scalar_tensor_tensor(
                                f[:, :], r[:, :], 1.0, sn[:, :],
                                mybir.AluOpType.subtract, mybir.AluOpType.mult,
                            )
                            fout.append((p, f))
                    if "out" in parts and "epi" in parts:
                        # Outs split across both rings, emitted after the
                        # whole epilogue so no doorbell-wait sits ahead of a
                        # compute op in an engine's stream. SP's ring is idle
                        # again by the time the first f is ready.
                        for p, f in fout:
                            if chain_opt and isinstance(f, tuple):
                                # split pair-3 outs; SP ring is idle and its
                                # SDMA moves to the next DMA's data phase
                                # while the previous write receipt is in
                                # flight, so the two receipts overlap
                                nc.sync.dma_start(
                                    out=out_d[:, p, 0:512], in_=f[0][:, :]
                                )
                                nc.sync.dma_start(
                                    out=out_d[:, p, 512:1024], in_=f[1][:, :]
                                )
                                continue
                            if chain_opt:
                                # p0-p2 on the SP ring (idle after the x
                                # feed; its sequencer just idle-waits on
                                # each f), p3 on ACT so the last out starts
                                # the moment f3 lands without queueing
                                # behind out2's drain.
                                eng = nc.sync if p < 3 else nc.scalar
                            elif out_eng == "act":
                                eng = nc.scalar
                            else:
                                eng = nc.sync if p < 2 else nc.scalar
                            eng.dma_start(out=out_d[:, p, :], in_=f[:, :])
    # Run the Bacc pass pipeline (wait splitting, reg alloc, ...) now; the
    # axon/pjrt execute path binds the primitive without finalizing.
    nc.finalize()
    return nc


def _np_bf16(a):
    import ml_dtypes

    return np.ascontiguousarray(a.astype(ml_dtypes.bfloat16))


def _prep_weights(W, b):
    """[16,16,64,3,3] capsule weights -> [128, 576] lhsT blocks (pre-summed
    over capsules, /16 for the uniform routing probs, duplicated into both
    partition halves; per (tap, half) a [64,32] block zero-masked outside
    cols 16*half..16*half+16).  Bias -> [128, 1] per-partition vector."""
    Wsum = np.asarray(W, dtype=np.float32).sum(axis=0) / 16.0  # [16, 64, 3, 3]
    w_arr = np.zeros((128, 576), np.float32)
    for t in range(9):
        kh, kw = divmod(t, 3)
        blk = np.ascontiguousarray(Wsum[:, :, kh, kw].T)  # [64 in, 16 out]
        for half in range(2):
            c0 = 32 * (2 * t + half) + 16 * half
            w_arr[0:64, c0 : c0 + 16] = blk
            w_arr[64:128, c0 : c0 + 16] = blk
    bsum = np.asarray(b, dtype=np.float32).sum(axis=0) / 16.0  # [16]
    bvec = np.zeros((128, 1), np.float32)
    for g in range(8):
        bvec[16 * g : 16 * g + 16, 0] = bsum
    return w_arr, bvec


def make_in_maps(x, W, b, conv_bf16=False):
    x = np.ascontiguousarray(np.asarray(x, dtype=np.float32))
    w_arr, bvec = _prep_weights(W, b)
    if conv_bf16:
        x = _np_bf16(x)
        w_arr = _np_bf16(w_arr)
    return [
        {
            "x": np.ascontiguousarray(
                x[c * B_PER_CORE : (c + 1) * B_PER_CORE].reshape(512, 66, 66)
            ),
            "w": w_arr,
            "bvec": bvec,
        }
        for c in range(N_CORES)
    ]


def gather_out(per_core_outs, chain_opt=False):
    """Unshuffle raw [128, 4, 1024] per-core dumps into [64, 65536, 1] f32.

    partition = 32*j + 16*half + o; free = (pair p, 512*rg + 64*hr + w);
    h-tile = 2j+half (baseline) or j+4*half (chain_opt);
    out[b=2p+rg, o*4096 + ht*512 + 64*hr + w]."""
    full = np.empty((64, 65536), np.float32)
    for c, raw in enumerate(per_core_outs):
        r = np.asarray(raw, dtype=np.float32).reshape(4, 2, 16, 4, 2, 8, 64)
        if chain_opt:
            # axes: [j, half, o, p, rg, hr, w] -> [p, rg, o, half, j, hr, w]
            v = r.transpose(3, 4, 2, 1, 0, 5, 6)
        else:
            # axes: [j, half, o, p, rg, hr, w] -> [p, rg, o, j, half, hr, w]
            v = r.transpose(3, 4, 2, 0, 1, 5, 6)
        full[c * 8 : (c + 1) * 8] = v.reshape(8, 65536)
    return full.reshape(64, 65536, 1)


KERNEL_OPTS = dict(
    conv_bf16=True,
    chain_opt=True,
    epi_mode="dve",
    ham_warm=8,
)


def kernel(x, W, b):
    from concourse.bass_utils import run_bass_kernel_spmd

    nc = _build_nc(**KERNEL_OPTS)
    in_maps = make_in_maps(x, W, b, conv_bf16=True)
    res = run_bass_kernel_spmd(nc, in_maps, list(range(N_CORES)))
    return gather_out(
        [res.results[c]["out"] for c in range(N_CORES)],
        chain_opt=KERNEL_OPTS.get("chain_opt", False),
    )

